# revision 36
# baseline (speedup 1.0000x reference)
"""Trainium2 Bass kernel for LlamaRALAAttention (B=2, S=4096, HID=2048, NH=16, NKV=4, HD=128).

Sharding: 8 cores = DP(batch=2) x TP(kv-head groups=4). Core c handles batch c//4,
kv group c%4 (4 q heads + 1 kv head). o_proj partials summed on host.

fp8 DoubleRow strategy (cost model: DR fp8 = 0.5 cyc/out-col, K=256/instr = 4x bf16):
  q proj:   1-pass fp8 (noise dilutes through Qg-mean and the positive rank-1-ish
            outer contraction; verified in numpy precision sim).
  k/v, phi: 3-pass fp8 (x8@W8 + dx8@W8 + x8@dW8) -> better than bf16 accuracy at
            0.75x bf16 PE cost. dx8/dW8 are fp8 residuals (no extra scaling needed,
            fp8 exponent covers them).
  o proj:   3-pass fp8 with on-chip ctx hi/lo fp8 split.
  Value-path storage fp16 (Kk, v, phi, outer); QkT fp8 (diluted like q).
Scales (powers of 2, folded into tables/drain scales/host):
  x*16, W*64 -> psum q/k = 1024x (rope tables carry 1/1024); v drain 1/1024.
  alpha carries 1/16 (fp16 outer range); phi drain carries 16/2^17; ctx stored
  = ctx/2^17 (fp8 range); out drain *2048 restores.
Layouts: q/phi/result/ctx in [d,s]; k/v in [s,d] (rope on free dim, alpha per-
  partition); KkT via PE transpose for logits matvecs.
"""

import sys

sys.path.insert(0, "/opt/trn_rl_repo")

import numpy as np
import ml_dtypes

import concourse.bass as bass
import concourse.mybir as mybir
import concourse.tile as tile
from concourse import bacc
from concourse.bass_utils import run_bass_kernel_spmd
from concourse.masks import make_identity

P = 128
S = 4096
HID = 2048
HD = 128
NHL = 4            # q heads per core
KO2 = 8            # 2048 / 256 contraction instrs per DR pass
CS = 512           # token chunk
NCH = S // CS      # 8
NST = S // P       # 32
ROPE_THETA = 10000.0

SX = 16.0          # x fp8 scale
SW = 64.0          # weight fp8 scale
SCTX = 131072.0    # ctx stored = ctx/SCTX (2^17)
SAL = 16.0         # alpha folded scale

F32 = mybir.dt.float32
F16 = mybir.dt.float16
BF16 = mybir.dt.bfloat16
F8 = mybir.dt.float8e4
NPF8 = ml_dtypes.float8_e4m3
NPH = np.float16
NPBF = ml_dtypes.bfloat16
DR = mybir.MatmulPerfMode.DoubleRow
AX = mybir.AxisListType.X
OP = mybir.AluOpType
ACT = mybir.ActivationFunctionType

_CACHE = {}


def _build():
    nc = bacc.Bacc("TRN2", target_bir_lowering=False, debug=False, num_devices=8)

    x8 = nc.dram_tensor("x8", [P, KO2, 2, S], F8, kind="ExternalInput").ap()
    dx8 = nc.dram_tensor("dx8", [P, KO2, 2, S], F8, kind="ExternalInput").ap()
    Wq8 = nc.dram_tensor("Wq8", [P, KO2, 2, NHL * HD], F8, kind="ExternalInput").ap()
    Wkv8 = nc.dram_tensor("Wkv8", [P, KO2, 2, 2 * HD], F8, kind="ExternalInput").ap()
    dWkv8 = nc.dram_tensor("dWkv8", [P, KO2, 2, 2 * HD], F8, kind="ExternalInput").ap()
    Wphi8 = nc.dram_tensor("Wphi8", [P, KO2, 2, NHL * HD], F8, kind="ExternalInput").ap()
    dWphi8 = nc.dram_tensor("dWphi8", [P, KO2, 2, NHL * HD], F8, kind="ExternalInput").ap()
    WoH8 = nc.dram_tensor("WoH8", [P, 2, 2, HID], F8, kind="ExternalInput").ap()
    WoL8 = nc.dram_tensor("WoL8", [P, 2, 2, HID], F8, kind="ExternalInput").ap()
    cosqT = nc.dram_tensor("cosqT", [P, S], F16, kind="ExternalInput").ap()
    sinqT = nc.dram_tensor("sinqT", [P, S], F16, kind="ExternalInput").ap()
    kcs = nc.dram_tensor("kcs", [P, NST, 2, HD], F16, kind="ExternalInput").ap()
    bphi_s = nc.dram_tensor("bphi_s", [P, NHL], F32, kind="ExternalInput").ap()
    RT = nc.dram_tensor("RT", [P, P], F16, kind="ExternalInput").ap()
    out = nc.dram_tensor("out", [P, NST, HID], BF16, kind="ExternalOutput").ap()

    from contextlib import ExitStack
    with tile.TileContext(nc) as tc, ExitStack() as es:
        res = es.enter_context(tc.tile_pool(name="res", bufs=1))
        wts = es.enter_context(tc.tile_pool(name="wts", bufs=1))
        xp = es.enter_context(tc.tile_pool(name="xp", bufs=2))
        dxp = es.enter_context(tc.tile_pool(name="dxp", bufs=2))
        tb = es.enter_context(tc.tile_pool(name="tb", bufs=2))
        st3 = es.enter_context(tc.tile_pool(name="st3", bufs=3))
        ctxp = es.enter_context(tc.tile_pool(name="ctxp", bufs=2))
        outp = es.enter_context(tc.tile_pool(name="outp", bufs=2))
        small = es.enter_context(tc.tile_pool(name="small", bufs=4))
        pq = es.enter_context(tc.tile_pool(name="pq", bufs=2, space="PSUM"))
        pr = es.enter_context(tc.tile_pool(name="pr", bufs=2, space="PSUM"))
        pphi = es.enter_context(tc.tile_pool(name="pphi", bufs=2, space="PSUM"))
        pmix = es.enter_context(tc.tile_pool(name="pmix", bufs=2, space="PSUM"))

        # ---- weights / tables (kv first; the rest stream in during chunk 0) ----
        Wkv_sb = wts.tile([P, KO2, 2, 2 * HD], F8)
        nc.sync.dma_start(Wkv_sb[:], Wkv8)
        dWkv_sb = wts.tile([P, KO2, 2, 2 * HD], F8)
        nc.sync.dma_start(dWkv_sb[:], dWkv8)
        RT_sb = res.tile([P, P], F16)
        nc.sync.dma_start(RT_sb[:], RT)
        bphi_sb = res.tile([P, NHL], F32)
        nc.sync.dma_start(bphi_sb[:], bphi_s)
        Wq_sb = wts.tile([P, KO2, 2, NHL * HD], F8)
        Wphi_sb = wts.tile([P, KO2, 2, NHL * HD], F8)
        dWphi_sb = wts.tile([P, KO2, 2, NHL * HD], F8)
        WoH_sb = wts.tile([P, 2, 2, HID], F8)
        WoL_sb = wts.tile([P, 2, 2, HID], F8)

        id16 = res.tile([P, P], F16)
        make_identity(nc, id16[:])
        idf32 = res.tile([P, P], F32)
        make_identity(nc, idf32[:])
        ones_f32 = res.tile([P, 1], F32)
        nc.vector.memset(ones_f32[:], 1.0)
        onesr_f32 = res.tile([1, P], F32)
        nc.vector.memset(onesr_f32[:], 1.0)
        negr_f32 = res.tile([1, P], F32)
        nc.vector.memset(negr_f32[:], -1.0)

        # ---- residents ----
        QkT8 = res.tile([P, NHL, S], F8)          # kappa(rope(q)), [d,s], fp8
        phiT16 = res.tile([P, NHL, S], F16)       # phi*SAL/SCTX, [d,s]
        Kk16 = res.tile([P, NST, HD], F16)        # kappa(rope(k)), [s,d]
        KkT16 = res.tile([P, S], F16)             # [d,s]
        v16 = res.tile([P, NST, HD], F16)         # [s,d]
        outer16 = res.tile([P, NHL, HD], F16)     # outer/SAL, [d,f]
        qg_parts = res.tile([P, NHL, NCH], F32)
        logits_sd = res.tile([P, NST, NHL], F32)
        alpha_sd = res.tile([P, NHL, NST], F32)   # alpha/SAL

        # ================= phase A: q/k/v/phi projections + rope + kappa =================
        for c in range(NCH):
            sl = slice(c * CS, (c + 1) * CS)
            xt = xp.tile([P, KO2, 2, CS], F8, tag="x")
            dxt = dxp.tile([P, KO2, 2, CS], F8, tag="dx")
            if c == 0:
                # startup: split x loads so the first kv matmuls start sooner
                nc.sync.dma_start(xt[:, :4], x8[:, :4, :, sl])
                nc.sync.dma_start(xt[:, 4:], x8[:, 4:, :, sl])
                nc.sync.dma_start(dxt[:, :4], dx8[:, :4, :, sl])
                nc.sync.dma_start(dxt[:, 4:], dx8[:, 4:, :, sl])
            else:
                nc.sync.dma_start(xt[:], x8[:, :, :, sl])
                nc.sync.dma_start(dxt[:], dx8[:, :, :, sl])
            cq = tb.tile([P, CS], F16, tag="cq")
            nc.sync.dma_start(cq[:], cosqT[:, sl])
            sq = tb.tile([P, CS], F16, tag="sq")
            nc.sync.dma_start(sq[:], sinqT[:, sl])
            kct = tb.tile([P, 4, 2, HD], F16, tag="kc")
            nc.sync.dma_start(kct[:], kcs[:, c * 4:(c + 1) * 4, :, :])
            if c == 0:
                nc.sync.dma_start(Wq_sb[:], Wq8)
                nc.sync.dma_start(Wphi_sb[:], Wphi8)
                nc.sync.dma_start(dWphi_sb[:], dWphi8)

            # ---- q (1-pass fp8 DR) + phi (3-pass), [d,s] ----
            # Emission order software-pipelines PE: q(h) -> [phi(h-1)] -> rope(h-1)
            # so the Act/DVE chain after each q-proj never stalls the PE.
            def q_proj(h):
                hsl = slice(h * HD, (h + 1) * HD)
                psq = pq.tile([P, CS], F32, tag="q", name=f"psq{h}")
                for n2 in range(2):
                    nsl = slice(n2 * 256, (n2 + 1) * 256)
                    for ko in range(KO2):
                        nc.tensor.matmul(
                            psq[:, nsl], Wq_sb[:, ko, :, hsl], xt[:, ko, :, nsl],
                            start=(ko == 0), stop=(ko == KO2 - 1), perf_mode=DR)
                q16 = st3.tile([P, CS], F16, tag="q16", name=f"q16_{h}")
                nc.scalar.activation(q16[:], psq[:], ACT.Identity)
                qs = st3.tile([P, CS], F16, tag="qs", name=f"qs{h}")
                nc.vector.tensor_mul(qs[:], q16[:], sq[:])
                qro = st3.tile([P, CS], F16, tag="qro", name=f"qro{h}")
                nc.vector.tensor_mul(qro[:], q16[:], cq[:])
                return qs, qro

            def q_rope(h, qs, qro):
                psr = pr.tile([P, CS], F32, tag="r", name=f"psr{h}")
                nc.tensor.matmul(psr[:], RT_sb[:], qs[:], start=True, stop=True)
                xr = st3.tile([P, CS], F16, tag="xr", name=f"xr{h}")
                nc.vector.tensor_add(xr[:], qro[:], psr[:])
                ea = st3.tile([P, CS], F16, tag="ea", name=f"ea{h}")
                nc.scalar.activation(ea[:], xr[:], ACT.Exp)
                tq = st3.tile([P, CS], F16, tag="tq", name=f"tq{h}")
                nc.vector.tensor_scalar_min(tq[:], ea[:], 1.0)
                nc.vector.scalar_tensor_tensor(
                    QkT8[:, h, sl], xr[:], 0.0, tq[:], OP.max, OP.add)
                nc.vector.tensor_reduce(
                    qg_parts[:, h, c:c + 1], QkT8[:, h, sl], AX, OP.add)

            def phi_proj(h):
                hsl = slice(h * HD, (h + 1) * HD)
                psp = pphi.tile([P, CS], F32, tag="p", name=f"psp{h}")
                passes = [(xt, Wphi_sb), (xt, dWphi_sb), (dxt, Wphi_sb)]
                for n2 in range(2):
                    nsl = slice(n2 * 256, (n2 + 1) * 256)
                    n = 0
                    for lt, rt in passes:
                        for ko in range(KO2):
                            nc.tensor.matmul(
                                psp[:, nsl], rt[:, ko, :, hsl], lt[:, ko, :, nsl],
                                start=(n == 0), stop=(n == 3 * KO2 - 1), perf_mode=DR)
                            n += 1
                nc.scalar.activation(phiT16[:, h, sl], psp[:], ACT.Identity,
                                     bias=bphi_sb[:, h:h + 1],
                                     scale=SAL / (SX * SW * SCTX))

            qp0 = None
            # ---- k/v (3-pass fp8 DR), [s,d] ----
            for st in range(4):
                stg = c * 4 + st
                ssl = slice(st * P, (st + 1) * P)
                pskv = pmix.tile([P, 2 * HD], F32, tag="mix")
                passes = [(xt, Wkv_sb), (xt, dWkv_sb), (dxt, Wkv_sb)]
                n = 0
                for lt, rt in passes:
                    for ko in range(KO2):
                        nc.tensor.matmul(
                            pskv[:], lt[:, ko, :, ssl], rt[:, ko, :, :],
                            start=(n == 0), stop=(n == 3 * KO2 - 1), perf_mode=DR)
                        n += 1
                k16 = st3.tile([P, HD], F16, tag="k16")
                nc.scalar.activation(k16[:], pskv[:, :HD], ACT.Identity)
                nc.scalar.activation(v16[:, stg, :], pskv[:, HD:], ACT.Identity,
                                     scale=1.0 / (SX * SW))
                # rope-k on free dim halves (tables carry 1/1024)
                kr = st3.tile([P, HD], F16, tag="kr")
                nc.vector.tensor_mul(kr[:], k16[:], kct[:, st, 0, :])
                t2 = st3.tile([P, 64], F16, tag="t2")
                nc.vector.tensor_mul(t2[:], k16[:, 64:], kct[:, st, 1, :64])
                nc.vector.tensor_sub(kr[:, :64], kr[:, :64], t2[:])
                t3 = st3.tile([P, 64], F16, tag="t3")
                nc.vector.tensor_mul(t3[:], k16[:, :64], kct[:, st, 1, 64:])
                nc.vector.tensor_add(kr[:, 64:], kr[:, 64:], t3[:])
                # kappa = max(x,0) + min(exp(x),1)
                ek = st3.tile([P, HD], F16, tag="ek")
                nc.scalar.activation(ek[:], kr[:], ACT.Exp)
                tk = st3.tile([P, HD], F16, tag="tk")
                nc.gpsimd.tensor_scalar_min(tk[:], ek[:], 1.0)
                nc.vector.scalar_tensor_tensor(
                    Kk16[:, stg, :], kr[:], 0.0, tk[:], OP.max, OP.add)

            if qp0 is None:
                qp0 = q_proj(0)
            # KkT transposes (PE) here: Kk16 for early s-tiles is ready by now
            for st in range(4):
                stg = c * 4 + st
                pst = pr.tile([P, P], F16, tag="r", name=f"pst{st}")
                nc.tensor.transpose(pst[:], Kk16[:, stg, :], id16[:])
                nc.vector.tensor_copy(KkT16[:, stg * P:(stg + 1) * P], pst[:])
            qp1 = q_proj(1)
            phi_proj(0)
            q_rope(0, *qp0)
            qp2 = q_proj(2)
            phi_proj(1)
            q_rope(1, *qp1)
            qp3 = q_proj(3)
            phi_proj(2)
            q_rope(2, *qp2)
            q_rope(3, *qp3)
            phi_proj(3)
            if c == 0:
                nc.sync.dma_start(WoH_sb[:], WoH8)
                nc.sync.dma_start(WoL_sb[:], WoL8)

        # ================= phase B: Qg, logits, softmax, outer =================
        qg_f = small.tile([P, NHL], F32, tag="qgf")
        for h in range(NHL):
            nc.vector.tensor_reduce(qg_f[:, h:h + 1], qg_parts[:, h, :], AX, OP.add)
        qg16 = small.tile([P, NHL], F16, tag="qg16")
        nc.vector.tensor_scalar_mul(qg16[:], qg_f[:], 1.0 / S)

        psl = pr.tile([P, NST, NHL], F32, tag="r")
        for st in range(NST):
            nc.tensor.matmul(psl[:, st, :], KkT16[:, st * P:(st + 1) * P],
                             qg16[:], start=True, stop=True)
        nc.vector.tensor_copy(logits_sd[:], psl[:])

        from concourse import bass_isa

        def softmax_head(h):
            lg = logits_sd[:, :, h]                       # [128, 32] stride NHL
            pmax = small.tile([P, 1], F32, tag="pmax", name=f"pmax{h}")
            nc.vector.tensor_reduce(pmax[:], lg, AX, OP.max)
            gmax = small.tile([P, 1], F32, tag="gmax", name=f"gmax{h}")
            nc.gpsimd.partition_all_reduce(gmax[:], pmax[:], 128, bass_isa.ReduceOp.max)
            ngm = small.tile([P, 1], F32, tag="ngm", name=f"ngm{h}")
            nc.vector.tensor_scalar_mul(ngm[:], gmax[:], -1.0)
            e_sd = small.tile([P, NST], F32, tag="esd", name=f"esd{h}")
            srow = small.tile([P, 1], F32, tag="srow", name=f"srow{h}")
            nc.scalar.activation(e_sd[:], lg, ACT.Exp, bias=ngm[:], accum_out=srow[:])
            stot = small.tile([P, 1], F32, tag="stot", name=f"stot{h}")
            nc.gpsimd.partition_all_reduce(stot[:], srow[:], 128, bass_isa.ReduceOp.add)
            rcpb = small.tile([P, 1], F32, tag="rcpb", name=f"rcpb{h}")
            nc.vector.reciprocal(rcpb[:], stot[:])
            nc.vector.tensor_scalar(
                alpha_sd[:, h, :], e_sd[:], rcpb[:], float(S) / SAL,
                OP.mult, OP.mult)

        def outer_head(h):
            pso = pq.tile([P, HD], F32, tag="q", name=f"pso{h}")
            for st in range(NST):
                kka = st3.tile([P, HD], F16, tag="kka", name=f"kka{h}_{st}")
                if st % 4 == 3:
                    nc.gpsimd.tensor_scalar_mul(
                        kka[:], Kk16[:, st, :], alpha_sd[:, h, st:st + 1])
                else:
                    nc.vector.tensor_scalar_mul(
                        kka[:], Kk16[:, st, :], alpha_sd[:, h, st:st + 1])
                nc.tensor.matmul(pso[:], kka[:], v16[:, st, :],
                                 start=(st == 0), stop=(st == NST - 1))
            nc.scalar.activation(outer16[:, h, :], pso[:], ACT.Identity)

        softmax_head(0)
        softmax_head(1)
        outer_head(0)
        softmax_head(2)
        outer_head(1)
        softmax_head(3)
        outer_head(2)
        outer_head(3)

        # ================= phase C: result, ctx hi/lo, o_proj =================
        # ctx for chunk c+1 is emitted before o_proj(c): its DVE/Act chain runs
        # in the shadow of o_proj(c)'s 16 PE groups.
        dr_engine = 0

        def ctx_chunk(c):
            sl = slice(c * CS, (c + 1) * CS)
            ctxh = ctxp.tile([P, NHL, CS], F8, tag="ch", name=f"ctxh{c}")
            ctxl = ctxp.tile([P, NHL, CS], F8, tag="cl", name=f"ctxl{c}")
            for h in range(NHL):
                psr = pr.tile([P, CS], F32, tag="r", name=f"psrc{h}")
                nc.tensor.matmul(psr[:], outer16[:, h, :], QkT8[:, h, sl],
                                 start=True, stop=True)
                cx = st3.tile([P, CS], F16, tag="cx", name=f"cx{h}")
                nc.vector.tensor_mul(cx[:], phiT16[:, h, sl], psr[:])
                nc.scalar.activation(ctxh[:, h, :], cx[:], ACT.Identity)
                nc.vector.scalar_tensor_tensor(
                    ctxl[:, h, :], ctxh[:, h, :], -1.0, cx[:], OP.mult, OP.add)
            return ctxh, ctxl

        ctx_cur = ctx_chunk(0)
        for c in range(NCH):
            ctxh, ctxl = ctx_cur
            if c + 1 < NCH:
                ctx_next = ctx_chunk(c + 1)
            for st in range(4):
                stg = c * 4 + st
                ssl = slice(st * P, (st + 1) * P)
                ob = outp.tile([P, 4, CS], BF16, tag="ob")
                for nq in range(4):
                    opool = (st * 4 + nq) % 3
                    if opool == 0:
                        po = pq.tile([P, CS], F32, tag="q")
                    elif opool == 1:
                        po = pmix.tile([P, CS], F32, tag="mix")
                    else:
                        po = pphi.tile([P, CS], F32, tag="p")
                    passes = [(ctxh, WoH_sb), (ctxl, WoH_sb), (ctxh, WoL_sb)]
                    for n2 in range(2):
                        n = 0
                        for ct, wt in passes:
                            for hp in range(2):
                                nc.tensor.matmul(
                                    po[:, n2 * 256:(n2 + 1) * 256],
                                    ct[:, 2 * hp:2 * hp + 2, ssl],
                                    wt[:, hp, :, nq * CS + n2 * 256:nq * CS + (n2 + 1) * 256],
                                    start=(n == 0), stop=(n == 5), perf_mode=DR)
                                n += 1
                    if c == NCH - 1:
                        eng = dr_engine % 2      # last chunk: drain on both engines
                    else:
                        eng = 0 if dr_engine % 4 == 0 else 1
                    dr_engine += 1
                    if eng == 0:
                        nc.vector.tensor_scalar_mul(ob[:, nq, :], po[:], SCTX / SW)
                    else:
                        nc.scalar.activation(ob[:, nq, :], po[:], ACT.Identity, scale=SCTX / SW)
                    if c == NCH - 1 and st == 3:
                        # last s-tile: per-quarter DMA so the tail pipelines
                        nc.sync.dma_start(out[:, stg, nq * CS:(nq + 1) * CS], ob[:, nq, :])
                if not (c == NCH - 1 and st == 3):
                    nc.sync.dma_start(out[:, stg, :], ob[:])
            if c + 1 < NCH:
                ctx_cur = ctx_next

    nc.compile()
    return nc


def _host_prep(hidden_states, position_ids, Wq, Wk, Wv, Wo, Wphi, bphi):
    B = hidden_states.shape[0]

    def q8(a):
        return np.clip(a, -240, 240).astype(NPF8)

    def split8(a):  # fp8 hi + residual
        hi = q8(a)
        lo = q8(a - hi.astype(np.float32))
        return hi, lo

    def wlay(W, sc=True):  # [2048, M] -> [p, ko, 2, M]
        Wl = (W * SW).astype(np.float32) if sc else W
        return np.ascontiguousarray(
            Wl.reshape(KO2, 2, P, -1).transpose(2, 0, 1, 3))

    inv_freq = (1.0 / (ROPE_THETA ** (np.arange(0, HD, 2, dtype=np.float32) / HD))).astype(np.float32)
    Rm = np.zeros((P, P), dtype=np.float32)
    Rm[np.arange(64), np.arange(64) + 64] = -1.0
    Rm[np.arange(64) + 64, np.arange(64)] = 1.0
    RT_np = np.ascontiguousarray(Rm.T).astype(NPH)

    in_maps = []
    for b in range(B):
        freqs = position_ids[b].astype(np.float32)[:, None] * inv_freq[None, :]
        emb = np.concatenate([freqs, freqs], axis=1)          # [S, 128]
        cos_b = np.cos(emb) / (SX * SW)
        sin_b = np.sin(emb) / (SX * SW)
        cosqT_b = np.ascontiguousarray(cos_b.T).astype(NPH)
        sinqT_b = np.ascontiguousarray(sin_b.T).astype(NPH)
        # kcs[p, st, 0/1, d]
        kcs_b = np.ascontiguousarray(
            np.stack([cos_b.reshape(NST, P, HD), sin_b.reshape(NST, P, HD)],
                     axis=2).transpose(1, 0, 2, 3)).astype(NPH)
        xs = (hidden_states[b].T * SX).astype(np.float32)      # [HID, S]
        x8_full = q8(xs)
        dx8_full = q8(xs - x8_full.astype(np.float32))
        x8_b = np.ascontiguousarray(
            x8_full.reshape(KO2, 2, P, S).transpose(2, 0, 1, 3))
        dx8_b = np.ascontiguousarray(
            dx8_full.reshape(KO2, 2, P, S).transpose(2, 0, 1, 3))
        for g in range(4):
            sl4 = slice(g * 512, (g + 1) * 512)
            sl1 = slice(g * 128, (g + 1) * 128)
            Wq_l = q8(wlay(Wq[:, sl4]))
            Wkv_hi, Wkv_lo = split8(wlay(np.concatenate([Wk[:, sl1], Wv[:, sl1]], axis=1)))
            Wphi_hi, Wphi_lo = split8(wlay(Wphi[:, sl4]))
            # Wo [512, 2048] -> [p, hp, 2, n]
            Wo_l = (Wo[sl4, :] * SW).astype(np.float32).reshape(2, 2, P, HID).transpose(2, 0, 1, 3)
            Wo_hi, Wo_lo = split8(np.ascontiguousarray(Wo_l))
            in_maps.append({
                "x8": x8_b, "dx8": dx8_b,
                "Wq8": Wq_l, "Wkv8": Wkv_hi, "dWkv8": Wkv_lo,
                "Wphi8": Wphi_hi, "dWphi8": Wphi_lo,
                "WoH8": Wo_hi, "WoL8": Wo_lo,
                "cosqT": cosqT_b, "sinqT": sinqT_b, "kcs": kcs_b,
                "bphi_s": np.ascontiguousarray(
                    (bphi[sl4] * SAL / SCTX).astype(np.float32).reshape(NHL, P).T),
                "RT": RT_np,
            })
    return in_maps


def kernel(hidden_states, position_ids, Wq, Wk, Wv, Wo, Wphi, bphi, _trace=False):
    if "nc" not in _CACHE:
        _CACHE["nc"] = _build()
    nc = _CACHE["nc"]
    in_maps = _host_prep(np.asarray(hidden_states), np.asarray(position_ids),
                         np.asarray(Wq), np.asarray(Wk), np.asarray(Wv),
                         np.asarray(Wo), np.asarray(Wphi), np.asarray(bphi))
    res = run_bass_kernel_spmd(nc, in_maps, list(range(8)), trace=_trace)
    _CACHE["last_res"] = res
    B = hidden_states.shape[0]
    out = np.empty((B, S, HID), dtype=np.float32)
    for b in range(B):
        acc = res.results[b * 4 + 0]["out"].astype(np.float32)
        for g in range(1, 4):
            acc = acc + res.results[b * 4 + g]["out"].astype(np.float32)
        out[b] = acc.reshape(P, NST, HID).transpose(1, 0, 2).reshape(S, HID)
    return out


# revision 39
# speedup vs baseline: 1.0291x; 1.0291x over previous
"""Trainium2 Bass kernel for LlamaRALAAttention (B=2, S=4096, HID=2048, NH=16, NKV=4, HD=128).

Sharding: 8 cores = DP(batch=2) x TP(kv-head groups=4). Core c handles batch c//4,
kv group c%4 (4 q heads + 1 kv head). o_proj partials summed on host.

fp8 DoubleRow strategy (cost model: DR fp8 = 0.5 cyc/out-col, K=256/instr = 4x bf16):
  q proj:   1-pass fp8 (noise dilutes through Qg-mean and the positive rank-1-ish
            outer contraction; verified in numpy precision sim).
  k/v, phi: 3-pass fp8 (x8@W8 + dx8@W8 + x8@dW8) -> better than bf16 accuracy at
            0.75x bf16 PE cost. dx8/dW8 are fp8 residuals (no extra scaling needed,
            fp8 exponent covers them).
  o proj:   3-pass fp8 with on-chip ctx hi/lo fp8 split.
  Value-path storage fp16 (Kk, v, phi, outer); QkT fp8 (diluted like q).
Scales (powers of 2, folded into tables/drain scales/host):
  x*16, W*64 -> psum q/k = 1024x (rope tables carry 1/1024); v drain 1/1024.
  alpha carries 1/16 (fp16 outer range); phi drain carries 16/2^17; ctx stored
  = ctx/2^17 (fp8 range); out drain *2048 restores.
Layouts: q/phi/result/ctx in [d,s]; k/v in [s,d] (rope on free dim, alpha per-
  partition); KkT via PE transpose for logits matvecs.
"""

import sys

sys.path.insert(0, "/opt/trn_rl_repo")

import numpy as np
import ml_dtypes

import concourse.bass as bass
import concourse.mybir as mybir
import concourse.tile as tile
from concourse import bacc
from concourse.bass_utils import run_bass_kernel_spmd
from concourse.masks import make_identity

P = 128
S = 4096
HID = 2048
HD = 128
NHL = 4            # q heads per core
KO2 = 8            # 2048 / 256 contraction instrs per DR pass
CS = 512           # token chunk
NCH = S // CS      # 8
NST = S // P       # 32
ROPE_THETA = 10000.0

SX = 16.0          # x fp8 scale
SW = 64.0          # weight fp8 scale
SCTX = 131072.0    # ctx stored = ctx/SCTX (2^17)
SAL = 16.0         # alpha folded scale

F32 = mybir.dt.float32
F16 = mybir.dt.float16
BF16 = mybir.dt.bfloat16
F8 = mybir.dt.float8e4
NPF8 = ml_dtypes.float8_e4m3
NPH = np.float16
NPBF = ml_dtypes.bfloat16
DR = mybir.MatmulPerfMode.DoubleRow
AX = mybir.AxisListType.X
OP = mybir.AluOpType
ACT = mybir.ActivationFunctionType

_CACHE = {}


def _build():
    nc = bacc.Bacc("TRN2", target_bir_lowering=False, debug=False, num_devices=8)

    x8 = nc.dram_tensor("x8", [P, KO2, 2, S], F8, kind="ExternalInput").ap()
    dx8 = nc.dram_tensor("dx8", [P, KO2, 2, S], F8, kind="ExternalInput").ap()
    Wq8 = nc.dram_tensor("Wq8", [P, KO2, 2, NHL * HD], F8, kind="ExternalInput").ap()
    Wkv8 = nc.dram_tensor("Wkv8", [P, KO2, 2, 2 * HD], F8, kind="ExternalInput").ap()
    dWkv8 = nc.dram_tensor("dWkv8", [P, KO2, 2, 2 * HD], F8, kind="ExternalInput").ap()
    Wphi8 = nc.dram_tensor("Wphi8", [P, KO2, 2, NHL * HD], F8, kind="ExternalInput").ap()
    dWphi8 = nc.dram_tensor("dWphi8", [P, KO2, 2, NHL * HD], F8, kind="ExternalInput").ap()
    WoH8 = nc.dram_tensor("WoH8", [P, 2, 2, HID], F8, kind="ExternalInput").ap()
    WoL8 = nc.dram_tensor("WoL8", [P, 2, 2, HID], F8, kind="ExternalInput").ap()
    cosqT = nc.dram_tensor("cosqT", [P, S], F16, kind="ExternalInput").ap()
    sinqT = nc.dram_tensor("sinqT", [P, S], F16, kind="ExternalInput").ap()
    kcs = nc.dram_tensor("kcs", [P, NST, 2, HD], F16, kind="ExternalInput").ap()
    bphi_s = nc.dram_tensor("bphi_s", [P, NHL], F32, kind="ExternalInput").ap()
    RT = nc.dram_tensor("RT", [P, P], F16, kind="ExternalInput").ap()
    out = nc.dram_tensor("out", [P, NST, HID], BF16, kind="ExternalOutput").ap()

    from contextlib import ExitStack
    with tile.TileContext(nc) as tc, ExitStack() as es:
        res = es.enter_context(tc.tile_pool(name="res", bufs=1))
        wts = es.enter_context(tc.tile_pool(name="wts", bufs=1))
        xp = es.enter_context(tc.tile_pool(name="xp", bufs=2))
        dxp = es.enter_context(tc.tile_pool(name="dxp", bufs=2))
        tb = es.enter_context(tc.tile_pool(name="tb", bufs=2))
        st3 = es.enter_context(tc.tile_pool(name="st3", bufs=3))
        ctxp = es.enter_context(tc.tile_pool(name="ctxp", bufs=2))
        outp = es.enter_context(tc.tile_pool(name="outp", bufs=2))
        small = es.enter_context(tc.tile_pool(name="small", bufs=4))
        pq = es.enter_context(tc.tile_pool(name="pq", bufs=2, space="PSUM"))
        pr = es.enter_context(tc.tile_pool(name="pr", bufs=2, space="PSUM"))
        pphi = es.enter_context(tc.tile_pool(name="pphi", bufs=2, space="PSUM"))
        pmix = es.enter_context(tc.tile_pool(name="pmix", bufs=2, space="PSUM"))

        # ---- weights / tables (kv first; the rest stream in during chunk 0) ----
        Wkv_sb = wts.tile([P, KO2, 2, 2 * HD], F8)
        nc.sync.dma_start(Wkv_sb[:], Wkv8)
        dWkv_sb = wts.tile([P, KO2, 2, 2 * HD], F8)
        nc.sync.dma_start(dWkv_sb[:], dWkv8)
        RT_sb = res.tile([P, P], F16)
        nc.sync.dma_start(RT_sb[:], RT)
        bphi_sb = res.tile([P, NHL], F32)
        nc.sync.dma_start(bphi_sb[:], bphi_s)
        Wq_sb = wts.tile([P, KO2, 2, NHL * HD], F8)
        Wphi_sb = wts.tile([P, KO2, 2, NHL * HD], F8)
        dWphi_sb = wts.tile([P, KO2, 2, NHL * HD], F8)
        WoH_sb = wts.tile([P, 2, 2, HID], F8)
        WoL_sb = wts.tile([P, 2, 2, HID], F8)

        id16 = res.tile([P, P], F16)
        make_identity(nc, id16[:])
        idf32 = res.tile([P, P], F32)
        make_identity(nc, idf32[:])
        ones_f32 = res.tile([P, 1], F32)
        nc.vector.memset(ones_f32[:], 1.0)
        onesr_f32 = res.tile([1, P], F32)
        nc.vector.memset(onesr_f32[:], 1.0)
        negr_f32 = res.tile([1, P], F32)
        nc.vector.memset(negr_f32[:], -1.0)

        # ---- residents ----
        QkT8 = res.tile([P, NHL, S], F8)          # kappa(rope(q)), [d,s], fp8
        phiT16 = res.tile([P, NHL, S], F16)       # phi*SAL/SCTX, [d,s]
        Kk16 = res.tile([P, NST, HD], F16)        # kappa(rope(k)), [s,d]
        KkT16 = res.tile([P, S], F16)             # [d,s]
        v16 = res.tile([P, NST, HD], F16)         # [s,d]
        outer16 = res.tile([P, NHL, HD], F16)     # outer/SAL, [d,f]
        qg_parts = res.tile([P, NHL, NCH], F32)
        logits_sd = res.tile([P, NST, NHL], F32)
        alpha_sd = res.tile([P, NHL, NST], F32)   # alpha/SAL

        # ================= phase A: q/k/v/phi projections + rope + kappa =================
        for c in range(NCH):
            sl = slice(c * CS, (c + 1) * CS)
            xt = xp.tile([P, KO2, 2, CS], F8, tag="x")
            dxt = dxp.tile([P, KO2, 2, CS], F8, tag="dx")
            if c == 0:
                # startup: split x loads so the first kv matmuls start sooner
                nc.sync.dma_start(xt[:, :4], x8[:, :4, :, sl])
                nc.sync.dma_start(xt[:, 4:], x8[:, 4:, :, sl])
                nc.sync.dma_start(dxt[:, :4], dx8[:, :4, :, sl])
                nc.sync.dma_start(dxt[:, 4:], dx8[:, 4:, :, sl])
            else:
                nc.sync.dma_start(xt[:], x8[:, :, :, sl])
                nc.sync.dma_start(dxt[:], dx8[:, :, :, sl])
            cq = tb.tile([P, CS], F16, tag="cq")
            nc.sync.dma_start(cq[:], cosqT[:, sl])
            sq = tb.tile([P, CS], F16, tag="sq")
            nc.sync.dma_start(sq[:], sinqT[:, sl])
            kct = tb.tile([P, 4, 2, HD], F16, tag="kc")
            nc.sync.dma_start(kct[:], kcs[:, c * 4:(c + 1) * 4, :, :])
            if c == 0:
                nc.sync.dma_start(Wq_sb[:], Wq8)
                nc.sync.dma_start(Wphi_sb[:], Wphi8)
                nc.sync.dma_start(dWphi_sb[:], dWphi8)

            # ---- q (1-pass fp8 DR) + phi (3-pass), [d,s] ----
            # Emission order software-pipelines PE: q(h) -> [phi(h-1)] -> rope(h-1)
            # so the Act/DVE chain after each q-proj never stalls the PE.
            def q_proj(h):
                hsl = slice(h * HD, (h + 1) * HD)
                psq = pq.tile([P, CS], F32, tag="q", name=f"psq{h}")
                for n2 in range(2):
                    nsl = slice(n2 * 256, (n2 + 1) * 256)
                    for ko in range(KO2):
                        nc.tensor.matmul(
                            psq[:, nsl], Wq_sb[:, ko, :, hsl], xt[:, ko, :, nsl],
                            start=(ko == 0), stop=(ko == KO2 - 1), perf_mode=DR)
                q16 = st3.tile([P, CS], F16, tag="q16", name=f"q16_{h}")
                nc.scalar.activation(q16[:], psq[:], ACT.Identity)
                qs = st3.tile([P, CS], F16, tag="qs", name=f"qs{h}")
                nc.vector.tensor_mul(qs[:], q16[:], sq[:])
                qro = st3.tile([P, CS], F16, tag="qro", name=f"qro{h}")
                nc.vector.tensor_mul(qro[:], q16[:], cq[:])
                return qs, qro

            def q_rope(h, qs, qro):
                psr = pr.tile([P, CS], F32, tag="r", name=f"psr{h}")
                nc.tensor.matmul(psr[:], RT_sb[:], qs[:], start=True, stop=True)
                xr = st3.tile([P, CS], F16, tag="xr", name=f"xr{h}")
                nc.vector.tensor_add(xr[:], qro[:], psr[:])
                ea = st3.tile([P, CS], F16, tag="ea", name=f"ea{h}")
                nc.scalar.activation(ea[:], xr[:], ACT.Exp)
                tq = st3.tile([P, CS], F16, tag="tq", name=f"tq{h}")
                nc.vector.tensor_scalar_min(tq[:], ea[:], 1.0)
                nc.vector.scalar_tensor_tensor(
                    QkT8[:, h, sl], xr[:], 0.0, tq[:], OP.max, OP.add)
                nc.vector.tensor_reduce(
                    qg_parts[:, h, c:c + 1], QkT8[:, h, sl], AX, OP.add)

            def phi_proj(h):
                hsl = slice(h * HD, (h + 1) * HD)
                psp = pphi.tile([P, CS], F32, tag="p", name=f"psp{h}")
                passes = [(xt, Wphi_sb), (xt, dWphi_sb), (dxt, Wphi_sb)]
                for n2 in range(2):
                    nsl = slice(n2 * 256, (n2 + 1) * 256)
                    n = 0
                    for lt, rt in passes:
                        for ko in range(KO2):
                            nc.tensor.matmul(
                                psp[:, nsl], rt[:, ko, :, hsl], lt[:, ko, :, nsl],
                                start=(n == 0), stop=(n == 3 * KO2 - 1), perf_mode=DR)
                            n += 1
                nc.scalar.activation(phiT16[:, h, sl], psp[:], ACT.Identity,
                                     bias=bphi_sb[:, h:h + 1],
                                     scale=SAL / (SX * SW * SCTX))

            qp0 = None
            # ---- k/v (3-pass fp8 DR), [s,d] ----
            for st in range(4):
                stg = c * 4 + st
                ssl = slice(st * P, (st + 1) * P)
                pskv = pmix.tile([P, 2 * HD], F32, tag="mix")
                passes = [(xt, Wkv_sb), (xt, dWkv_sb), (dxt, Wkv_sb)]
                n = 0
                for lt, rt in passes:
                    for ko in range(KO2):
                        nc.tensor.matmul(
                            pskv[:], lt[:, ko, :, ssl], rt[:, ko, :, :],
                            start=(n == 0), stop=(n == 3 * KO2 - 1), perf_mode=DR)
                        n += 1
                k16 = st3.tile([P, HD], F16, tag="k16")
                nc.scalar.activation(k16[:], pskv[:, :HD], ACT.Identity)
                nc.scalar.activation(v16[:, stg, :], pskv[:, HD:], ACT.Identity,
                                     scale=1.0 / (SX * SW))
                # rope-k on free dim halves (tables carry 1/1024)
                kr = st3.tile([P, HD], F16, tag="kr")
                nc.vector.tensor_mul(kr[:], k16[:], kct[:, st, 0, :])
                t2 = st3.tile([P, 64], F16, tag="t2")
                nc.vector.tensor_mul(t2[:], k16[:, 64:], kct[:, st, 1, :64])
                nc.vector.tensor_sub(kr[:, :64], kr[:, :64], t2[:])
                t3 = st3.tile([P, 64], F16, tag="t3")
                nc.vector.tensor_mul(t3[:], k16[:, :64], kct[:, st, 1, 64:])
                nc.vector.tensor_add(kr[:, 64:], kr[:, 64:], t3[:])
                # kappa = max(x,0) + min(exp(x),1)
                ek = st3.tile([P, HD], F16, tag="ek")
                nc.scalar.activation(ek[:], kr[:], ACT.Exp)
                tk = st3.tile([P, HD], F16, tag="tk")
                nc.gpsimd.tensor_scalar_min(tk[:], ek[:], 1.0)
                nc.vector.scalar_tensor_tensor(
                    Kk16[:, stg, :], kr[:], 0.0, tk[:], OP.max, OP.add)

            if qp0 is None:
                qp0 = q_proj(0)
            # KkT transposes (PE) here: Kk16 for early s-tiles is ready by now
            for st in range(4):
                stg = c * 4 + st
                pst = pr.tile([P, P], F16, tag="r", name=f"pst{st}")
                nc.tensor.transpose(pst[:], Kk16[:, stg, :], id16[:])
                nc.vector.tensor_copy(KkT16[:, stg * P:(stg + 1) * P], pst[:])
            qp1 = q_proj(1)
            phi_proj(0)
            q_rope(0, *qp0)
            qp2 = q_proj(2)
            phi_proj(1)
            q_rope(1, *qp1)
            qp3 = q_proj(3)
            phi_proj(2)
            q_rope(2, *qp2)
            q_rope(3, *qp3)
            phi_proj(3)
            if c == 0:
                nc.sync.dma_start(WoH_sb[:], WoH8)
                nc.sync.dma_start(WoL_sb[:], WoL8)

        # ================= phase B: Qg, logits, softmax, outer =================
        qg_f = small.tile([P, NHL], F32, tag="qgf")
        for h in range(NHL):
            nc.vector.tensor_reduce(qg_f[:, h:h + 1], qg_parts[:, h, :], AX, OP.add)
        qg16 = small.tile([P, NHL], F16, tag="qg16")
        nc.vector.tensor_scalar_mul(qg16[:], qg_f[:], 1.0 / S)

        psl = pr.tile([P, NST, NHL], F32, tag="r")
        for st in range(NST):
            nc.tensor.matmul(psl[:, st, :], KkT16[:, st * P:(st + 1) * P],
                             qg16[:], start=True, stop=True)
        nc.vector.tensor_copy(logits_sd[:], psl[:])

        from concourse import bass_isa

        def softmax_head(h):
            lg = logits_sd[:, :, h]                       # [128, 32] stride NHL
            pmax = small.tile([P, 1], F32, tag="pmax", name=f"pmax{h}")
            nc.vector.tensor_reduce(pmax[:], lg, AX, OP.max)
            gmax = small.tile([P, 1], F32, tag="gmax", name=f"gmax{h}")
            nc.gpsimd.partition_all_reduce(gmax[:], pmax[:], 128, bass_isa.ReduceOp.max)
            ngm = small.tile([P, 1], F32, tag="ngm", name=f"ngm{h}")
            nc.vector.tensor_scalar_mul(ngm[:], gmax[:], -1.0)
            e_sd = small.tile([P, NST], F32, tag="esd", name=f"esd{h}")
            srow = small.tile([P, 1], F32, tag="srow", name=f"srow{h}")
            nc.scalar.activation(e_sd[:], lg, ACT.Exp, bias=ngm[:], accum_out=srow[:])
            stot = small.tile([P, 1], F32, tag="stot", name=f"stot{h}")
            nc.gpsimd.partition_all_reduce(stot[:], srow[:], 128, bass_isa.ReduceOp.add)
            rcpb = small.tile([P, 1], F32, tag="rcpb", name=f"rcpb{h}")
            nc.vector.reciprocal(rcpb[:], stot[:])
            nc.vector.tensor_scalar(
                alpha_sd[:, h, :], e_sd[:], rcpb[:], float(S) / SAL,
                OP.mult, OP.mult)

        def outer_head(h):
            pso = pq.tile([P, HD], F32, tag="q", name=f"pso{h}")
            for st in range(NST):
                kka = st3.tile([P, HD], F16, tag="kka", name=f"kka{h}_{st}")
                if st % 4 == 3:
                    nc.gpsimd.tensor_scalar_mul(
                        kka[:], Kk16[:, st, :], alpha_sd[:, h, st:st + 1])
                else:
                    nc.vector.tensor_scalar_mul(
                        kka[:], Kk16[:, st, :], alpha_sd[:, h, st:st + 1])
                nc.tensor.matmul(pso[:], kka[:], v16[:, st, :],
                                 start=(st == 0), stop=(st == NST - 1))
            nc.scalar.activation(outer16[:, h, :], pso[:], ACT.Identity)

        softmax_head(0)
        softmax_head(1)
        outer_head(0)
        softmax_head(2)
        outer_head(1)
        softmax_head(3)
        outer_head(2)
        outer_head(3)

        # ================= phase C: result, ctx hi/lo, o_proj =================
        # ctx for chunk c+1 is emitted before o_proj(c): its DVE/Act chain runs
        # in the shadow of o_proj(c)'s 16 PE groups.
        dr_engine = 0

        def ctx_chunk(c):
            sl = slice(c * CS, (c + 1) * CS)
            ctxh = ctxp.tile([P, NHL, CS], F8, tag="ch", name=f"ctxh{c}")
            ctxl = ctxp.tile([P, NHL, CS], F8, tag="cl", name=f"ctxl{c}")
            for h in range(NHL):
                psr = pr.tile([P, CS], F32, tag="r", name=f"psrc{h}")
                nc.tensor.matmul(psr[:], outer16[:, h, :], QkT8[:, h, sl],
                                 start=True, stop=True)
                cx = st3.tile([P, CS], F16, tag="cx", name=f"cx{h}")
                nc.vector.tensor_mul(cx[:], phiT16[:, h, sl], psr[:])
                nc.scalar.activation(ctxh[:, h, :], cx[:], ACT.Identity)
                nc.vector.scalar_tensor_tensor(
                    ctxl[:, h, :], ctxh[:, h, :], -1.0, cx[:], OP.mult, OP.add)
            return ctxh, ctxl

        ctx_cur = ctx_chunk(0)
        for c in range(NCH):
            ctxh, ctxl = ctx_cur
            if c + 1 < NCH:
                ctx_next = ctx_chunk(c + 1)
            for st in range(4):
                stg = c * 4 + st
                ssl = slice(st * P, (st + 1) * P)
                obA = outp.tile([P, 2, CS], BF16, tag="obA")
                obB = outp.tile([P, 2, CS], BF16, tag="obB")
                for nq in range(4):
                    opool = (st * 4 + nq) % 3
                    if opool == 0:
                        po = pq.tile([P, CS], F32, tag="q")
                    elif opool == 1:
                        po = pmix.tile([P, CS], F32, tag="mix")
                    else:
                        po = pphi.tile([P, CS], F32, tag="p")
                    passes = [(ctxh, WoH_sb), (ctxl, WoH_sb), (ctxh, WoL_sb)]
                    for n2 in range(2):
                        n = 0
                        for ct, wt in passes:
                            for hp in range(2):
                                nc.tensor.matmul(
                                    po[:, n2 * 256:(n2 + 1) * 256],
                                    ct[:, 2 * hp:2 * hp + 2, ssl],
                                    wt[:, hp, :, nq * CS + n2 * 256:nq * CS + (n2 + 1) * 256],
                                    start=(n == 0), stop=(n == 5), perf_mode=DR)
                                n += 1
                    ob = obA if nq < 2 else obB
                    if c == NCH - 1:
                        eng = dr_engine % 2      # last chunk: drain on both engines
                    else:
                        eng = 0 if dr_engine % 4 == 0 else 1
                    dr_engine += 1
                    if eng == 0:
                        nc.vector.tensor_scalar_mul(ob[:, nq % 2, :], po[:], SCTX / SW)
                    else:
                        nc.scalar.activation(ob[:, nq % 2, :], po[:], ACT.Identity, scale=SCTX / SW)
                    if nq == 1:
                        nc.sync.dma_start(out[:, stg, :2 * CS], obA[:])
                    elif nq == 3:
                        nc.sync.dma_start(out[:, stg, 2 * CS:], obB[:])
            if c + 1 < NCH:
                ctx_cur = ctx_next

    nc.compile()
    return nc


def _host_prep(hidden_states, position_ids, Wq, Wk, Wv, Wo, Wphi, bphi):
    B = hidden_states.shape[0]

    def q8(a):
        return np.clip(a, -240, 240).astype(NPF8)

    def split8(a):  # fp8 hi + residual
        hi = q8(a)
        lo = q8(a - hi.astype(np.float32))
        return hi, lo

    def wlay(W, sc=True):  # [2048, M] -> [p, ko, 2, M]
        Wl = (W * SW).astype(np.float32) if sc else W
        return np.ascontiguousarray(
            Wl.reshape(KO2, 2, P, -1).transpose(2, 0, 1, 3))

    inv_freq = (1.0 / (ROPE_THETA ** (np.arange(0, HD, 2, dtype=np.float32) / HD))).astype(np.float32)
    Rm = np.zeros((P, P), dtype=np.float32)
    Rm[np.arange(64), np.arange(64) + 64] = -1.0
    Rm[np.arange(64) + 64, np.arange(64)] = 1.0
    RT_np = np.ascontiguousarray(Rm.T).astype(NPH)

    in_maps = []
    for b in range(B):
        freqs = position_ids[b].astype(np.float32)[:, None] * inv_freq[None, :]
        emb = np.concatenate([freqs, freqs], axis=1)          # [S, 128]
        cos_b = np.cos(emb) / (SX * SW)
        sin_b = np.sin(emb) / (SX * SW)
        cosqT_b = np.ascontiguousarray(cos_b.T).astype(NPH)
        sinqT_b = np.ascontiguousarray(sin_b.T).astype(NPH)
        # kcs[p, st, 0/1, d]
        kcs_b = np.ascontiguousarray(
            np.stack([cos_b.reshape(NST, P, HD), sin_b.reshape(NST, P, HD)],
                     axis=2).transpose(1, 0, 2, 3)).astype(NPH)
        xs = (hidden_states[b].T * SX).astype(np.float32)      # [HID, S]
        x8_full = q8(xs)
        dx8_full = q8(xs - x8_full.astype(np.float32))
        x8_b = np.ascontiguousarray(
            x8_full.reshape(KO2, 2, P, S).transpose(2, 0, 1, 3))
        dx8_b = np.ascontiguousarray(
            dx8_full.reshape(KO2, 2, P, S).transpose(2, 0, 1, 3))
        for g in range(4):
            sl4 = slice(g * 512, (g + 1) * 512)
            sl1 = slice(g * 128, (g + 1) * 128)
            Wq_l = q8(wlay(Wq[:, sl4]))
            Wkv_hi, Wkv_lo = split8(wlay(np.concatenate([Wk[:, sl1], Wv[:, sl1]], axis=1)))
            Wphi_hi, Wphi_lo = split8(wlay(Wphi[:, sl4]))
            # Wo [512, 2048] -> [p, hp, 2, n]
            Wo_l = (Wo[sl4, :] * SW).astype(np.float32).reshape(2, 2, P, HID).transpose(2, 0, 1, 3)
            Wo_hi, Wo_lo = split8(np.ascontiguousarray(Wo_l))
            in_maps.append({
                "x8": x8_b, "dx8": dx8_b,
                "Wq8": Wq_l, "Wkv8": Wkv_hi, "dWkv8": Wkv_lo,
                "Wphi8": Wphi_hi, "dWphi8": Wphi_lo,
                "WoH8": Wo_hi, "WoL8": Wo_lo,
                "cosqT": cosqT_b, "sinqT": sinqT_b, "kcs": kcs_b,
                "bphi_s": np.ascontiguousarray(
                    (bphi[sl4] * SAL / SCTX).astype(np.float32).reshape(NHL, P).T),
                "RT": RT_np,
            })
    return in_maps


def kernel(hidden_states, position_ids, Wq, Wk, Wv, Wo, Wphi, bphi, _trace=False):
    if "nc" not in _CACHE:
        _CACHE["nc"] = _build()
    nc = _CACHE["nc"]
    in_maps = _host_prep(np.asarray(hidden_states), np.asarray(position_ids),
                         np.asarray(Wq), np.asarray(Wk), np.asarray(Wv),
                         np.asarray(Wo), np.asarray(Wphi), np.asarray(bphi))
    res = run_bass_kernel_spmd(nc, in_maps, list(range(8)), trace=_trace)
    _CACHE["last_res"] = res
    B = hidden_states.shape[0]
    out = np.empty((B, S, HID), dtype=np.float32)
    for b in range(B):
        acc = res.results[b * 4 + 0]["out"].astype(np.float32)
        for g in range(1, 4):
            acc = acc + res.results[b * 4 + g]["out"].astype(np.float32)
        out[b] = acc.reshape(P, NST, HID).transpose(1, 0, 2).reshape(S, HID)
    return out
